# revision 2
# baseline (speedup 1.0000x reference)
"""CVRP decoder kernel for 8 Trainium2 NeuronCores (pure batch data-parallel).

Self-contained: hardcodes shapes B=64,N=256,M=1000,S=500,E=128,H=8,D=16 and
shards the batch 8-per-core. Host-side prep is layout-only (transposes, pads,
dtype casts, constant augmentation); all model math runs on device.

Host layouts fed to the device (per core shard of BLOC=8 instances):
  elnT  [BLOC,128,256] f32   encoded_last_node transposed (e on partitions)
  loadv [BLOC,256]     f32   load row
  ktm   [BLOC,128,1000]bf16  k with (h,d) stacked on partitions (hd, m)
  kts   [BLOC,128,500] bf16  k_s likewise
  vam   [BLOC,128,8,256]bf16 v in 32-wide head bands [v_h | ones | 0] per
                             l-tile (l = t*128 + p), ready as att stationary
  vas   [BLOC,128,4,256]bf16 v_s likewise (S padded to 512)
  m01m  [BLOC,128,8,256]bf16 multiplicative 0/1 ninf mask, transposed (l, n)
  m01s  [BLOC,128,4,256]bf16 sols mask likewise
  shkb  [BLOC,128,1000]bf16  single_head_key (e, m) natural layout
  (+ small shared weights wqT/wql/wc0/wc1/sel/blk/onesc/ones1r)

Device per instance (software-pipelined A+B1 | C(prev) | B2):
  q:     qT = WqT^T@elnT + wql@load (PE,f32r) -> bf16 -> 8 Pool copies spread
         it into the block-diagonal qtz so per-head scores come from stacked
         K=128 matmuls (operands must sit at partition base 0 on this stack)
  MHA:   per l-tile: scoreT = ktm-tile^T @ qtz (bf16, 4x512 cols) -> ACT
         exp(x/4) -> DVE mask-multiply (m01 broadcast across 8 heads) ->
         8 att matmuls [32-band] accumulating over l-tiles; the ones column
         of vam yields softmax denominators for free; normalization via
         sel/blk indicator matmuls + DVE reciprocal, all in att layout
  comb:  cmbT = sum_c wc_c @ mh_half (PSUM f32) -> bf16
  final: transposed: per m-tile scoreT = shk-tile^T @ cmbT -> ACT tanh ->
         ACT exp(10x) -> DVE multiply by the SAME m01m tiles -> den row via
         ones-stationary matmuls -> reciprocal broadcast by K=1 matmul ->
         DVE scale -> bf16 outT; host un-transposes and casts to f32.

Environment workarounds: TileContext drain split onto single-wait NOPs and a
global one-wait-per-instruction legalization pass (this walrus rejects >1
sync wait per instruction).
"""

import re
from contextlib import ExitStack

import numpy as np

import concourse.bass as bass
import concourse.mybir as mybir
import concourse.tile as tile

# ---------------------------------------------------------------- constants
B, N, M, S, E, H, D = 64, 256, 1000, 500, 128, 8, 16
SQRT_E = 11.313708498984761
CLIP = 10.0
NCORES = 8
BLOC = B // NCORES  # 8 batch instances per core
MT, ST = 8, 4      # l-tiles (128-wide) covering M and S
MP, SP = 128 * MT, 128 * ST

FP32 = mybir.dt.float32
F32R = mybir.dt.float32r
BF16 = mybir.dt.bfloat16
AF = mybir.ActivationFunctionType

M_TILES = [(i * 128, min(128, M - i * 128)) for i in range(MT)]
S_TILES = [(i * 128, min(128, S - i * 128)) for i in range(ST)]


def _r(ap):
    """view an fp32 AP as float32r (bitwise; used on DRAM/DMA side)"""
    return ap.bitcast(F32R)


# ------------------------------------------------- tile drain-split patch
# This walrus build rejects >1 sync-wait on a Drain ("Too many sync wait
# commands"), so split the kernel-tail global-clock waits onto single-wait
# NOPs preceding the drain.
def _patch_tile_drain():
    from bass_rust import ScopedClock, VectorClock

    def _drain_and_barrier(self, tick_clock, wait_clock):
        gc = tick_clock.global_clock
        vals = [int(x) for x in re.findall(r"\d+", repr(gc))]
        for proc, tick in enumerate(vals):
            if tick > 0:
                partial = VectorClock()
                partial.require_at_least(proc, tick)
                nop = self.nc.sync.nop(nofuse=True, hint="split_drain_wait")
                wait_clock.add_sem_waits(nop.ins, ScopedClock({None: partial}))
        self.nc.sync.drain()  # waits covered by the NOPs above
        self.nc.all_engine_barrier()
        assert self.sems is not None
        popped = self.nc._tile_sem_poison_stack.pop()
        assert popped is self._sem_poison
        self.nc.clear_and_free_semaphores(list(self.sems.allocated().values()))
        self.nc.all_engine_barrier()

    tile.TileContext._drain_and_barrier = _drain_and_barrier


_patch_tile_drain()


def _legalize_single_waits(nc):
    """This walrus build accepts at most ONE sync-wait per instruction; hoist
    extra waits onto single-wait NOP carriers placed just before, on the same
    engine (engines execute in order, so the gate is preserved)."""
    n_multi_upd = 0
    for f in nc.m.functions:
        for bb in f.blocks:
            out = []
            for inst in bb.instructions:
                si = inst.sync_info
                if si is not None and len(si.on_wait) > 1:
                    waits = list(si.on_wait)
                    si.on_wait = waits[-1:]
                    for w in waits[:-1]:
                        nop = mybir.InstNoOp(
                            name=nc.get_next_instruction_name(), ins=[], outs=[])
                        nop.engine = inst.engine
                        nop.sync_info = mybir.SyncInfo(on_wait=[w], on_update=[])
                        out.append(nop)
                if si is not None and len(si.on_update) > 1:
                    n_multi_upd += 1
                out.append(inst)
            bb.instructions = out
    if n_multi_upd:
        print(f"WARNING: {n_multi_upd} instructions with >1 sync updates")


def build_nc(legalize=True):
    nc = bass.Bass(trn_type="TRN2", target_bir_lowering=False, debug=False)

    t = {}
    t["elnT"] = nc.dram_tensor("elnT", [BLOC, E, N], FP32, kind="ExternalInput").ap()
    t["loadv"] = nc.dram_tensor("loadv", [BLOC, N], FP32, kind="ExternalInput").ap()
    t["ktm"] = nc.dram_tensor("ktm", [BLOC, 128, M], BF16, kind="ExternalInput").ap()
    t["kts"] = nc.dram_tensor("kts", [BLOC, 128, S], BF16, kind="ExternalInput").ap()
    t["vam"] = nc.dram_tensor("vam", [BLOC, 128, MT, 256], BF16, kind="ExternalInput").ap()
    t["vas"] = nc.dram_tensor("vas", [BLOC, 128, ST, 256], BF16, kind="ExternalInput").ap()
    t["m01m"] = nc.dram_tensor("m01m", [BLOC, 128, MT, 256], BF16, kind="ExternalInput").ap()
    t["m01s"] = nc.dram_tensor("m01s", [BLOC, 128, ST, 256], BF16, kind="ExternalInput").ap()
    t["shkb"] = nc.dram_tensor("shkb", [BLOC, E, M], BF16, kind="ExternalInput").ap()
    t["wqT"] = nc.dram_tensor("wqT", [E, E], FP32, kind="ExternalInput").ap()
    t["wql"] = nc.dram_tensor("wql", [1, E], FP32, kind="ExternalInput").ap()
    t["wc0"] = nc.dram_tensor("wc0", [E, E], BF16, kind="ExternalInput").ap()
    t["wc1"] = nc.dram_tensor("wc1", [E, E], BF16, kind="ExternalInput").ap()
    t["sel"] = nc.dram_tensor("sel", [128, 4], BF16, kind="ExternalInput").ap()
    t["blk"] = nc.dram_tensor("blk", [4, 128], BF16, kind="ExternalInput").ap()
    t["onesc"] = nc.dram_tensor("onesc", [128, 1], BF16, kind="ExternalInput").ap()
    t["ones1r"] = nc.dram_tensor("ones1r", [1, 128], BF16, kind="ExternalInput").ap()
    t["outT"] = nc.dram_tensor("outT", [BLOC, 128, MT, 256], BF16,
                               kind="ExternalOutput").ap()

    with ExitStack() as ctx:
        tc = ctx.enter_context(tile.TileContext(nc))
        build_kernel(ctx, tc, t)
    if legalize:
        _legalize_single_waits(nc)
    return nc


def build_kernel(ctx, tc, t):
    nc = tc.nc
    ctx.enter_context(nc.allow_low_precision("bf16/fp32r matmuls + bf16 io"))

    singles = ctx.enter_context(tc.tile_pool(name="singles", bufs=1))
    sb_in = ctx.enter_context(tc.tile_pool(name="sb_in", bufs=2))
    sb_u = ctx.enter_context(tc.tile_pool(name="sb_u", bufs=3))
    sb_misc = ctx.enter_context(tc.tile_pool(name="sb_misc", bufs=2))
    ps_score = ctx.enter_context(tc.tile_pool(name="ps_score", bufs=2, space="PSUM"))
    ps_att = ctx.enter_context(tc.tile_pool(name="ps_att", bufs=2, space="PSUM"))
    ps_small = ctx.enter_context(tc.tile_pool(name="ps_small", bufs=2, space="PSUM"))

    def small_ps():
        return ps_small.tile([128, 512], FP32, name="ps", tag="ps")

    def att_ps_tile():
        return ps_att.tile([128, 512], FP32, name="attps", tag="attps")

    # ---------------- once-per-kernel weights ----------------
    wqT = singles.tile([E, E], F32R)
    nc.sync.dma_start(out=wqT, in_=_r(t["wqT"]))
    wql = singles.tile([1, E], F32R)
    nc.sync.dma_start(out=wql, in_=_r(t["wql"]))
    wc = []
    for c in range(2):
        w = singles.tile([E, E], BF16, name=f"wc{c}", tag=f"wc{c}")
        nc.sync.dma_start(out=w, in_=t[f"wc{c}"])
        wc.append(w)
    sel = singles.tile([128, 4], BF16)
    nc.sync.dma_start(out=sel, in_=t["sel"])
    blk = singles.tile([4, 128], BF16)
    nc.sync.dma_start(out=blk, in_=t["blk"])
    onesc = singles.tile([128, 1], BF16)
    nc.sync.dma_start(out=onesc, in_=t["onesc"])
    ones1r = singles.tile([1, 128], BF16)
    nc.sync.dma_start(out=ones1r, in_=t["ones1r"])

    # persistent block-diagonal q tiles (zero blocks never rewritten)
    qtz_slots = [singles.tile([128, H * 256], BF16, name=f"qtz{i}",
                              tag=f"qtz{i}") for i in range(2)]
    for q in qtz_slots:
        nc.vector.memset(q, 0.0)

    # ---------------- per batch instance ----------------
    def phase_a_b1(b):
        """loads + q projection + MHA pass over M"""
        elnT = sb_in.tile([E, N], F32R, tag="elnT")
        nc.sync.dma_start(out=elnT, in_=_r(t["elnT"][b]))
        loadv = sb_in.tile([1, N], F32R, tag="loadv")
        lrow = bass.AP(tensor=t["loadv"].tensor, offset=t["loadv"].offset + b * N,
                       ap=[[0, 1], [1, N]])
        nc.sync.dma_start(out=loadv, in_=_r(lrow))
        ktm = sb_in.tile([128, M], BF16, tag="ktm")
        nc.sync.dma_start(out=ktm, in_=t["ktm"][b])
        kts = sb_in.tile([128, S], BF16, tag="kts")
        nc.sync.dma_start(out=kts, in_=t["kts"][b])
        vam = sb_in.tile([128, MT, 256], BF16, tag="vam")
        nc.sync.dma_start(out=vam, in_=t["vam"][b])
        vas = sb_in.tile([128, ST, 256], BF16, tag="vas")
        nc.sync.dma_start(out=vas, in_=t["vas"][b])
        m01m = sb_in.tile([128, MT, 256], BF16, tag="m01m")
        nc.sync.dma_start(out=m01m, in_=t["m01m"][b])
        m01s = sb_in.tile([128, ST, 256], BF16, tag="m01s")
        nc.sync.dma_start(out=m01s, in_=t["m01s"][b])
        shkb = sb_in.tile([E, M], BF16, tag="shkb")
        nc.sync.dma_start(out=shkb, in_=t["shkb"][b])

        # q projection -> bf16 -> spread into block-diagonal qtz
        qT_ps = small_ps()
        nc.tensor.matmul(qT_ps[:, 0:N], wqT, elnT, start=True, stop=False)
        nc.tensor.matmul(qT_ps[:, 0:N], wql, loadv, start=False, stop=True)
        qTb = sb_misc.tile([128, N], BF16, tag="qTb")
        nc.vector.tensor_copy(qTb, qT_ps[:, 0:N])
        qtz = qtz_slots[b % 2]
        for h in range(H):
            nc.gpsimd.tensor_copy(qtz[16 * h:16 * h + 16, h * 256:(h + 1) * 256],
                                  qTb[16 * h:16 * h + 16, :])

        mhc1 = mha_pass(b, ktm, vam, m01m, M_TILES, qtz, "m1")
        return (b, mhc1, kts, vas, m01s, shkb, m01m)

    def mha_pass(b, kt, va, m01, ltiles, qtz, tagp):
        att = att_ps_tile()
        nlt = len(ltiles)
        for lt, (l0, L) in enumerate(ltiles):
            u = sb_u.tile([128, H, 256], BF16, tag=f"u_{tagp}")
            for p in range(2):
                sc = ps_score.tile([128, 1024], FP32, tag="sc")
                for j in range(2):
                    nc.tensor.matmul(sc[0:L, j * 512:(j + 1) * 512],
                                     kt[:, l0:l0 + L],
                                     qtz[:, (4 * p + 2 * j) * 256:
                                         (4 * p + 2 * j + 2) * 256],
                                     start=True, stop=True)
                nc.scalar.activation(u[0:L, 4 * p:4 * p + 4, :], sc[0:L, :],
                                     AF.Exp, scale=0.25)
            # mask multiply, m01 l-tile broadcast across the 8 heads
            msl = m01[0:L, lt, :]
            mb = bass.AP(tensor=msl.tensor, offset=msl.offset,
                         ap=[msl.ap[0], [0, H], [1, 256]])
            nc.vector.tensor_mul(u[0:L], u[0:L], mb)
            for h in range(H):
                nc.tensor.matmul(att[32 * (h % 4):32 * (h % 4) + 32,
                                     (h // 4) * 256:(h // 4 + 1) * 256],
                                 va[0:L, lt, 32 * h:32 * h + 32],
                                 u[0:L, h, :],
                                 start=(lt == 0), stop=(lt == nlt - 1),
                                 tile_position=(0, 32 * (h % 4)),
                                 skip_group_check=True)
        # normalize: den rows (32i+16) -> reciprocal -> broadcast -> multiply
        attc = sb_misc.tile([128, 512], BF16, tag=f"attc_{tagp}")
        nc.vector.tensor_copy(attc, att)
        den_ps = small_ps()
        nc.tensor.matmul(den_ps[0:4, 0:512], sel, attc, start=True, stop=True)
        den_r = sb_misc.tile([4, 512], BF16, tag=f"denr_{tagp}")
        nc.vector.reciprocal(den_r, den_ps[0:4, 0:512])
        rb_ps = small_ps()
        nc.tensor.matmul(rb_ps[:, 0:512], blk, den_r, start=True, stop=True)
        mhc = sb_misc.tile([128, 512], BF16, tag=f"mhc_{tagp}")
        nc.vector.tensor_mul(mhc, attc, rb_ps[:, 0:512])
        return mhc

    def phase_b2(st):
        b, mhc1, kts, vas, m01s, shkb, m01m = st
        mhc2 = mha_pass(b, kts, vas, m01s, S_TILES, qtz_slots[b % 2], "m2")
        mh = sb_misc.tile([128, 512], BF16, tag="mh")
        nc.vector.tensor_add(mh, mhc1, mhc2)
        cmb_ps = small_ps()
        for c in range(2):
            nc.tensor.matmul(cmb_ps[:, 0:256], wc[c],
                             mh[:, c * 256:(c + 1) * 256],
                             start=(c == 0), stop=(c == 1))
        cmbT = sb_misc.tile([128, 256], BF16, tag="cmbT")
        nc.vector.tensor_copy(cmbT, cmb_ps[:, 0:256])
        return (b, cmbT, shkb, m01m)

    def phase_c(st):
        b, cmbT, shkb, m01m = st
        w = sb_u.tile([128, MT, 256], BF16, tag="w")
        dtot = att_ps_tile()  # rows 0:1 used; shares the att PSUM ring
        for g in range(4):
            fs = small_ps()
            for j in range(2):
                m0, L = M_TILES[2 * g + j]
                nc.tensor.matmul(fs[0:L, j * 256:(j + 1) * 256],
                                 shkb[:, m0:m0 + L], cmbT,
                                 start=True, stop=True)
            th = sb_misc.tile([128, 512], BF16, tag="th")
            nc.scalar.activation(th, fs[:, 0:512], AF.Tanh,
                                 scale=float(1.0 / SQRT_E))
            nc.scalar.activation(w[:, 2 * g:2 * g + 2, :], th, AF.Exp,
                                 scale=float(CLIP))
            nc.vector.tensor_mul(w[:, 2 * g:2 * g + 2, :],
                                 w[:, 2 * g:2 * g + 2, :],
                                 m01m[:, 2 * g:2 * g + 2, :])
            for j in range(2):
                m0, L = M_TILES[2 * g + j]
                nc.tensor.matmul(dtot[0:1, 0:256], onesc[0:L, :],
                                 w[0:L, 2 * g + j, :],
                                 start=(g == 0 and j == 0),
                                 stop=(g == 3 and j == 1),
                                 skip_group_check=True)
        den_r = sb_misc.tile([1, 256], BF16, tag="denr_f")
        nc.vector.reciprocal(den_r, dtot[0:1, 0:256])
        rbf_ps = small_ps()
        nc.tensor.matmul(rbf_ps[:, 0:256], ones1r, den_r, start=True, stop=True)
        rbf = sb_misc.tile([128, 256], BF16, tag="rbf")
        nc.vector.tensor_copy(rbf, rbf_ps[:, 0:256])
        oT = sb_u.tile([128, MT, 256], BF16, tag="oT")
        rb_b = bass.AP(tensor=rbf.tensor, offset=rbf.offset,
                       ap=[rbf.ap[0], [0, MT], [1, 256]])
        nc.vector.tensor_mul(oT, w, rb_b)
        nc.sync.dma_start(out=t["outT"][b], in_=oT)

    st = phase_b2(phase_a_b1(0))
    for b in range(1, BLOC):
        half = phase_a_b1(b)
        phase_c(st)
        st = phase_b2(half)
    phase_c(st)


# ------------------------------------------------------------- host prep
def _host_arrays(inputs):
    """Layout-only host prep: transposes, pads, casts, constant columns."""
    f32 = np.float32
    eln = np.asarray(inputs["encoded_last_node"], f32)      # [B,N,E]
    load = np.asarray(inputs["load"], f32)                   # [B,N]
    solm = np.asarray(inputs["sols_mask_pomo"], f32)         # [B,N,S]
    ninf = np.asarray(inputs["ninf_mask"], f32)              # [B,N,M]
    k = np.asarray(inputs["k"], f32)                         # [B,H,M,D]
    v = np.asarray(inputs["v"], f32)
    k_s = np.asarray(inputs["k_s"], f32)                     # [B,H,S,D]
    v_s = np.asarray(inputs["v_s"], f32)
    shk = np.asarray(inputs["single_head_key"], f32)         # [B,E,M]
    wq = np.asarray(inputs["Wq_last"], f32)                  # [E, E+1]
    wcm = np.asarray(inputs["W_combine"], f32)               # [E, E]

    def tile_lp(x, pad_to):  # [B, L, C] -> [B, 128, T, C] with l = t*128+p
        Bn, L, C = x.shape
        xp = np.zeros((Bn, pad_to, C), x.dtype)
        xp[:, :L] = x
        return np.ascontiguousarray(
            xp.reshape(Bn, pad_to // 128, 128, C).transpose(0, 2, 1, 3))

    def build_va(vv, pad_to):  # [B,H,L,D] -> banded [B,128,T,256]
        Bn, Hn, L, Dn = vv.shape
        va = np.zeros((Bn, L, Hn, 32), np.float32)
        va[:, :, :, :D] = vv.transpose(0, 2, 1, 3)
        va[:, :, :, D] = 1.0
        return tile_lp(va.reshape(Bn, L, Hn * 32), pad_to).astype(np.dtype("bfloat16"))

    bf16 = np.dtype("bfloat16")
    arrs = {
        "elnT": np.ascontiguousarray(eln.transpose(0, 2, 1)),
        "loadv": np.ascontiguousarray(load),
        "ktm": np.ascontiguousarray(
            k.transpose(0, 1, 3, 2).reshape(B, H * D, M)).astype(bf16),
        "kts": np.ascontiguousarray(
            k_s.transpose(0, 1, 3, 2).reshape(B, H * D, S)).astype(bf16),
        "vam": build_va(v, MP),
        "vas": build_va(v_s, SP),
        "m01m": tile_lp((ninf == 0).astype(np.float32).transpose(0, 2, 1),
                        MP).astype(bf16),
        "m01s": tile_lp((solm == 0).astype(np.float32).transpose(0, 2, 1),
                        SP).astype(bf16),
        "shkb": shk.astype(bf16),
    }
    arrs["wqT"] = np.ascontiguousarray(wq[:, :E].T)
    arrs["wql"] = np.ascontiguousarray(wq[:, E:E + 1].T)
    for c in range(2):
        w = np.zeros((E, E), np.float32)
        for i in range(4):
            w[32 * i:32 * i + 16, :] = wcm[:, 64 * c + 16 * i:64 * c + 16 * i + 16].T
        arrs[f"wc{c}"] = w.astype(bf16)
    selm = np.zeros((128, 4), np.float32)
    for i in range(4):
        selm[32 * i + 16, i] = 1.0
    arrs["sel"] = selm.astype(bf16)
    blkm = np.zeros((4, 128), np.float32)
    for j in range(4):
        blkm[j, 32 * j:32 * j + 16] = 1.0
    arrs["blk"] = blkm.astype(bf16)
    arrs["onesc"] = np.ones((128, 1), np.float32).astype(bf16)
    arrs["ones1r"] = np.ones((1, 128), np.float32).astype(bf16)
    return arrs


_SHARED = ("wqT", "wql", "wc0", "wc1", "sel", "blk", "onesc", "ones1r")


def _in_maps(inputs):
    arrs = _host_arrays(inputs)
    in_maps = []
    for c in range(NCORES):
        s = slice(c * BLOC, (c + 1) * BLOC)
        in_maps.append({n: (a if n in _SHARED else np.ascontiguousarray(a[s]))
                        for n, a in arrs.items()})
    return in_maps


def _untranspose(outT):
    """[B,128,MT,256] bf16 -> [B,N,M] f32"""
    o = np.asarray(outT).astype(np.float32)          # [B, p, t, n]
    o = o.transpose(0, 3, 2, 1).reshape(B, N, MP)    # [B, n, t*128+p]
    return np.ascontiguousarray(o[:, :, :M])


# ------------------------------------------------------------- entry point
_NC_CACHE = None


def kernel(**inputs):
    global _NC_CACHE
    from concourse.bass_utils import run_bass_kernel_spmd

    if _NC_CACHE is None:
        _NC_CACHE = build_nc()
    nc = _NC_CACHE
    res = run_bass_kernel_spmd(nc, _in_maps(inputs), core_ids=list(range(NCORES)))
    full = np.concatenate([res.results[c]["outT"] for c in range(NCORES)], axis=0)
    return _untranspose(full)


def bench(inputs, iters=24):
    """Device-resident repeated execution; returns min wall ns per launch
    (includes PJRT dispatch, excludes H2D of inputs)."""
    import time
    import jax
    import concourse.mybir as mb
    from concourse import bass2jax
    from jax.experimental.shard_map import shard_map
    from jax.sharding import Mesh, NamedSharding, PartitionSpec

    global _NC_CACHE
    if _NC_CACHE is None:
        _NC_CACHE = build_nc()
    nc = _NC_CACHE
    bass2jax.install_neuronx_cc_hook()

    partition_name = nc.partition_id_tensor.name if nc.partition_id_tensor else None
    in_names, out_names, out_avals, zero_outs = [], [], [], []
    for alloc in nc.m.functions[0].allocations:
        if not isinstance(alloc, mb.MemoryLocationSet):
            continue
        name = alloc.memorylocations[0].name
        if alloc.kind == "ExternalInput":
            if name != partition_name:
                in_names.append(name)
        elif alloc.kind == "ExternalOutput":
            shape = tuple(alloc.tensor_shape)
            dtype = mb.dt.np(alloc.dtype)
            out_names.append(name)
            out_avals.append(jax.core.ShapedArray(shape, dtype))
            zero_outs.append(np.zeros((NCORES * shape[0], *shape[1:]), dtype))
    n_params = len(in_names)
    n_outs = len(out_avals)
    all_names = in_names + out_names + ([partition_name] if partition_name else [])
    donate = tuple(range(n_params, n_params + n_outs))

    def _body(*args):
        operands = list(args)
        if partition_name is not None:
            operands.append(bass2jax.partition_id_tensor())
        return tuple(bass2jax._bass_exec_p.bind(
            *operands, out_avals=tuple(out_avals), in_names=tuple(all_names),
            out_names=tuple(out_names), lowering_input_output_aliases=(),
            sim_require_finite=True, sim_require_nnan=True, nc=nc))

    devices = jax.devices()[:NCORES]
    mesh = Mesh(np.asarray(devices), ("core",))
    sharded = jax.jit(
        shard_map(_body, mesh=mesh,
                  in_specs=(PartitionSpec("core"),) * (n_params + n_outs),
                  out_specs=(PartitionSpec("core"),) * n_outs, check_rep=False),
        donate_argnums=donate, keep_unused=True)

    in_maps = _in_maps(inputs)
    concat_in = [np.concatenate([np.asarray(in_maps[c][nm]) for c in range(NCORES)],
                                axis=0) if in_maps[0][nm].ndim > 2 or nm not in _SHARED
                 else np.asarray(in_maps[0][nm]) for nm in in_names]
    # shared weights are identical per core: replicate along core axis
    concat_in = []
    for nm in in_names:
        if nm in _SHARED:
            a = np.asarray(in_maps[0][nm])
            concat_in.append(np.concatenate([a] * NCORES, axis=0))
        else:
            concat_in.append(np.concatenate(
                [np.asarray(in_maps[c][nm]) for c in range(NCORES)], axis=0))
    sh = NamedSharding(mesh, PartitionSpec("core"))
    dev_in = [jax.device_put(a, sh) for a in concat_in]
    times = []
    for it in range(iters):
        dev_zeros = [jax.device_put(z, sh) for z in zero_outs]
        jax.block_until_ready(dev_zeros)
        t0 = time.perf_counter()
        outs = sharded(*dev_in, *dev_zeros)
        jax.block_until_ready(outs)
        times.append(time.perf_counter() - t0)
    print(f"  launch times (ms): {[round(t*1e3, 2) for t in times]}")
    return int(min(times[1:]) * 1e9) if len(times) > 1 else int(times[0] * 1e9)


if __name__ == "__main__":
    build_nc()
    print("build ok")


# revision 7
# speedup vs baseline: 2.0764x; 2.0764x over previous
"""CVRP decoder kernel for 8 Trainium2 NeuronCores (pure batch data-parallel).

Self-contained: hardcodes shapes B=64,N=256,M=1000,S=500,E=128,H=8,D=16 and
shards the batch 8-per-core. Host-side prep is layout-only (transposes, pads,
dtype casts, constant augmentation); all model math runs on device.

Host layouts fed to the device (per core shard of BLOC=8 instances):
  elnT  [BLOC,128,256] f32   encoded_last_node transposed (e on partitions)
  loadv [BLOC,256]     f32   load row
  ktm   [BLOC,128,1000]bf16  k with (h,d) stacked on partitions (hd, m)
  kts   [BLOC,128,500] bf16  k_s likewise
  vam   [BLOC,128,8,256]bf16 v in 32-wide head bands [v_h | ones | 0] per
                             l-tile (l = t*128 + p), ready as att stationary
  vas   [BLOC,128,4,256]bf16 v_s likewise (S padded to 512)
  m01m  [BLOC,128,8,256]bf16 multiplicative 0/1 ninf mask, transposed (l, n)
  m01s  [BLOC,128,4,256]bf16 sols mask likewise
  shkb  [BLOC,128,1000]bf16  single_head_key (e, m) natural layout
  (+ small shared weights wqT/wql/wc0/wc1/sel/blk/onesc/ones1r)

Device per instance (software-pipelined A+B1 | C(prev) | B2):
  q:     qT = WqT^T@elnT + wql@load (PE,f32r) -> bf16 -> 8 Pool copies spread
         it into the block-diagonal qtz so per-head scores come from stacked
         K=128 matmuls (operands must sit at partition base 0 on this stack)
  MHA:   per l-tile: scoreT = ktm-tile^T @ qtz (bf16, 4x512 cols) -> ACT
         exp(x/4) -> DVE mask-multiply (m01 broadcast across 8 heads) ->
         8 att matmuls [32-band] accumulating over l-tiles; the ones column
         of vam yields softmax denominators for free; normalization via
         sel/blk indicator matmuls + DVE reciprocal, all in att layout
  comb:  cmbT = sum_c wc_c @ mh_half (PSUM f32) -> bf16
  final: transposed: per m-tile scoreT = shk-tile^T @ cmbT -> ACT tanh ->
         ACT exp(10x) -> DVE multiply by the SAME m01m tiles -> den row via
         ones-stationary matmuls -> reciprocal broadcast by K=1 matmul ->
         DVE scale -> bf16 outT; host un-transposes and casts to f32.

Environment workarounds: TileContext drain split onto single-wait NOPs and a
global one-wait-per-instruction legalization pass (this walrus rejects >1
sync wait per instruction).
"""

import re
from contextlib import ExitStack

import numpy as np

import concourse.bass as bass
import concourse.mybir as mybir
import concourse.tile as tile

# ---------------------------------------------------------------- constants
B, N, M, S, E, H, D = 64, 256, 1000, 500, 128, 8, 16
SQRT_E = 11.313708498984761
CLIP = 10.0
NCORES = 8
BLOC = B // NCORES  # 8 batch instances per core
MT, ST = 8, 4      # l-tiles (128-wide) covering M and S
MP, SP = 128 * MT, 128 * ST

FP32 = mybir.dt.float32
F32R = mybir.dt.float32r
BF16 = mybir.dt.bfloat16
AF = mybir.ActivationFunctionType

M_TILES = [(i * 128, min(128, M - i * 128)) for i in range(MT)]
S_TILES = [(i * 128, min(128, S - i * 128)) for i in range(ST)]


def _r(ap):
    """view an fp32 AP as float32r (bitwise; used on DRAM/DMA side)"""
    return ap.bitcast(F32R)


# ------------------------------------------------- tile drain-split patch
# This walrus build rejects >1 sync-wait on a Drain ("Too many sync wait
# commands"), so split the kernel-tail global-clock waits onto single-wait
# NOPs preceding the drain.
def _patch_tile_drain():
    from bass_rust import ScopedClock, VectorClock

    def _drain_and_barrier(self, tick_clock, wait_clock):
        gc = tick_clock.global_clock
        vals = [int(x) for x in re.findall(r"\d+", repr(gc))]
        for proc, tick in enumerate(vals):
            if tick > 0:
                partial = VectorClock()
                partial.require_at_least(proc, tick)
                nop = self.nc.sync.nop(nofuse=True, hint="split_drain_wait")
                wait_clock.add_sem_waits(nop.ins, ScopedClock({None: partial}))
        self.nc.sync.drain()  # waits covered by the NOPs above
        self.nc.all_engine_barrier()
        assert self.sems is not None
        popped = self.nc._tile_sem_poison_stack.pop()
        assert popped is self._sem_poison
        self.nc.clear_and_free_semaphores(list(self.sems.allocated().values()))
        self.nc.all_engine_barrier()

    tile.TileContext._drain_and_barrier = _drain_and_barrier


_patch_tile_drain()


def _legalize_single_waits(nc):
    """This walrus build accepts at most ONE sync-wait per instruction; hoist
    extra waits onto single-wait NOP carriers placed just before, on the same
    engine (engines execute in order, so the gate is preserved)."""
    n_multi_upd = 0
    for f in nc.m.functions:
        for bb in f.blocks:
            out = []
            for inst in bb.instructions:
                si = inst.sync_info
                if si is not None and len(si.on_wait) > 1:
                    waits = list(si.on_wait)
                    si.on_wait = waits[-1:]
                    for w in waits[:-1]:
                        nop = mybir.InstNoOp(
                            name=nc.get_next_instruction_name(), ins=[], outs=[])
                        nop.engine = inst.engine
                        nop.sync_info = mybir.SyncInfo(on_wait=[w], on_update=[])
                        out.append(nop)
                if si is not None and len(si.on_update) > 1:
                    n_multi_upd += 1
                out.append(inst)
            bb.instructions = out
    if n_multi_upd:
        print(f"WARNING: {n_multi_upd} instructions with >1 sync updates")


def build_nc(legalize=True):
    nc = bass.Bass(trn_type="TRN2", target_bir_lowering=False, debug=False)

    t = {}
    t["elnT"] = nc.dram_tensor("elnT", [BLOC, E, N], FP32, kind="ExternalInput").ap()
    t["loadv"] = nc.dram_tensor("loadv", [BLOC, N], FP32, kind="ExternalInput").ap()
    t["ktm"] = nc.dram_tensor("ktm", [BLOC, 128, M], BF16, kind="ExternalInput").ap()
    t["kts"] = nc.dram_tensor("kts", [BLOC, 128, S], BF16, kind="ExternalInput").ap()
    t["vam"] = nc.dram_tensor("vam", [BLOC, 128, MT, 256], BF16, kind="ExternalInput").ap()
    t["vas"] = nc.dram_tensor("vas", [BLOC, 128, ST, 256], BF16, kind="ExternalInput").ap()
    t["m01m"] = nc.dram_tensor("m01m", [BLOC, 128, MT, 256], BF16, kind="ExternalInput").ap()
    t["m01s"] = nc.dram_tensor("m01s", [BLOC, 128, ST, 256], BF16, kind="ExternalInput").ap()
    t["shkb"] = nc.dram_tensor("shkb", [BLOC, E, MP], BF16, kind="ExternalInput").ap()
    t["wqT"] = nc.dram_tensor("wqT", [E, E], FP32, kind="ExternalInput").ap()
    t["wql"] = nc.dram_tensor("wql", [1, E], FP32, kind="ExternalInput").ap()
    t["wc0"] = nc.dram_tensor("wc0", [E, E], BF16, kind="ExternalInput").ap()
    t["wc1"] = nc.dram_tensor("wc1", [E, E], BF16, kind="ExternalInput").ap()
    t["sel"] = nc.dram_tensor("sel", [128, 4], FP32, kind="ExternalInput").ap()
    t["blk"] = nc.dram_tensor("blk", [4, 128], FP32, kind="ExternalInput").ap()
    t["onesc"] = nc.dram_tensor("onesc", [128, 1], FP32, kind="ExternalInput").ap()
    t["ones1r"] = nc.dram_tensor("ones1r", [1, 128], FP32, kind="ExternalInput").ap()
    t["outT"] = nc.dram_tensor("outT", [BLOC, 128, MT, 256], FP32,
                               kind="ExternalOutput").ap()

    with ExitStack() as ctx:
        tc = ctx.enter_context(tile.TileContext(nc))
        build_kernel(ctx, tc, t)
    if legalize:
        _legalize_single_waits(nc)
    return nc


def build_kernel(ctx, tc, t):
    nc = tc.nc
    ctx.enter_context(nc.allow_low_precision("bf16/fp32r matmuls + bf16 io"))

    singles = ctx.enter_context(tc.tile_pool(name="singles", bufs=1))
    sb_in = ctx.enter_context(tc.tile_pool(name="sb_in", bufs=2))
    sb_u = ctx.enter_context(tc.tile_pool(name="sb_u", bufs=3))
    sb_misc = ctx.enter_context(tc.tile_pool(name="sb_misc", bufs=2))
    ps_score = ctx.enter_context(tc.tile_pool(name="ps_score", bufs=2, space="PSUM"))
    ps_att = ctx.enter_context(tc.tile_pool(name="ps_att", bufs=2, space="PSUM"))
    ps_small = ctx.enter_context(tc.tile_pool(name="ps_small", bufs=2, space="PSUM"))

    def small_ps():
        return ps_small.tile([128, 512], FP32, name="ps", tag="ps")

    def att_ps_tile():
        return ps_att.tile([128, 512], FP32, name="attps", tag="attps")

    # ---------------- once-per-kernel weights ----------------
    wqT = singles.tile([E, E], F32R)
    nc.sync.dma_start(out=wqT, in_=_r(t["wqT"]))
    wql = singles.tile([1, E], F32R)
    nc.sync.dma_start(out=wql, in_=_r(t["wql"]))
    wc = []
    for c in range(2):
        w = singles.tile([E, E], BF16, name=f"wc{c}", tag=f"wc{c}")
        nc.sync.dma_start(out=w, in_=t[f"wc{c}"])
        wc.append(w)
    sel = singles.tile([128, 4], F32R)
    nc.sync.dma_start(out=sel, in_=_r(t["sel"]))
    blk = singles.tile([4, 128], F32R)
    nc.sync.dma_start(out=blk, in_=_r(t["blk"]))
    onesc = singles.tile([128, 1], F32R)
    nc.sync.dma_start(out=onesc, in_=_r(t["onesc"]))
    ones1r = singles.tile([1, 128], F32R)
    nc.sync.dma_start(out=ones1r, in_=_r(t["ones1r"]))

    # persistent block-diagonal q tiles (zero blocks never rewritten)
    qtz_slots = [singles.tile([128, H * 256], BF16, name=f"qtz{i}",
                              tag=f"qtz{i}") for i in range(2)]
    for q in qtz_slots:
        nc.vector.memset(q, 0.0)

    # ---------------- per batch instance ----------------
    def phase_a_b1(b):
        """loads + q projection + MHA pass over M"""
        elnT = sb_in.tile([E, N], F32R, tag="elnT")
        nc.sync.dma_start(out=elnT, in_=_r(t["elnT"][b]))
        loadv = sb_in.tile([1, N], F32R, tag="loadv")
        lrow = bass.AP(tensor=t["loadv"].tensor, offset=t["loadv"].offset + b * N,
                       ap=[[0, 1], [1, N]])
        nc.sync.dma_start(out=loadv, in_=_r(lrow))
        ktm = sb_in.tile([128, M], BF16, tag="ktm")
        nc.sync.dma_start(out=ktm, in_=t["ktm"][b])
        kts = sb_in.tile([128, S], BF16, tag="kts")
        nc.sync.dma_start(out=kts, in_=t["kts"][b])
        vam = sb_in.tile([128, MT, 256], BF16, tag="vam")
        nc.sync.dma_start(out=vam, in_=t["vam"][b])
        vas = sb_in.tile([128, ST, 256], BF16, tag="vas")
        nc.sync.dma_start(out=vas, in_=t["vas"][b])
        m01m = sb_in.tile([128, MT, 256], BF16, tag="m01m")
        nc.sync.dma_start(out=m01m, in_=t["m01m"][b])
        m01s = sb_in.tile([128, ST, 256], BF16, tag="m01s")
        nc.sync.dma_start(out=m01s, in_=t["m01s"][b])
        shkb = sb_in.tile([E, MP], BF16, tag="shkb")
        nc.sync.dma_start(out=shkb, in_=t["shkb"][b])

        # q projection -> bf16 -> spread into block-diagonal qtz
        qT_ps = small_ps()
        nc.tensor.matmul(qT_ps[:, 0:N], wqT, elnT, start=True, stop=False)
        nc.tensor.matmul(qT_ps[:, 0:N], wql, loadv, start=False, stop=True)
        qTb = sb_misc.tile([128, N], BF16, tag="qTb")
        nc.vector.tensor_copy(qTb, qT_ps[:, 0:N])
        qtz = qtz_slots[b % 2]
        for h in range(H):
            # engines need 32-aligned start partitions; DMA does not
            nc.sync.dma_start(out=qtz[16 * h:16 * h + 16, h * 256:(h + 1) * 256],
                              in_=qTb[16 * h:16 * h + 16, :])

        mhc1 = mha_pass(b, ktm, vam, m01m, M_TILES, qtz, "m1")
        return (b, mhc1, kts, vas, m01s, shkb, m01m)

    def mha_pass(b, kt, va, m01, ltiles, qtz, tagp):
        att = att_ps_tile()
        nlt = len(ltiles)
        for lt, (l0, L) in enumerate(ltiles):
            u = sb_u.tile([128, H, 256], BF16, tag=f"u_{tagp}")
            for p in range(2):
                sc = ps_score.tile([128, 1024], FP32, tag="sc")
                for j in range(2):
                    nc.tensor.matmul(sc[0:L, j * 512:(j + 1) * 512],
                                     kt[:, l0:l0 + L],
                                     qtz[:, (4 * p + 2 * j) * 256:
                                         (4 * p + 2 * j + 2) * 256],
                                     start=True, stop=True)
                nc.scalar.activation(u[0:L, 4 * p:4 * p + 4, :], sc[0:L, :],
                                     AF.Exp, scale=0.25)
            # mask multiply, m01 l-tile broadcast across the 8 heads
            msl = m01[0:L, lt, :]
            mb = bass.AP(tensor=msl.tensor, offset=msl.offset,
                         ap=[msl.ap[0], [0, H], [1, 256]])
            nc.vector.tensor_mul(u[0:L], u[0:L], mb)
            for h in range(H):
                nc.tensor.matmul(att[32 * (h % 4):32 * (h % 4) + 32,
                                     (h // 4) * 256:(h // 4 + 1) * 256],
                                 va[0:L, lt, 32 * h:32 * h + 32],
                                 u[0:L, h, :],
                                 start=(lt == 0), stop=(lt == nlt - 1),
                                 tile_position=(0, 32 * (h % 4)),
                                 skip_group_check=True)
        # normalize: den rows (32i+16) -> reciprocal -> broadcast -> multiply
        attc = sb_misc.tile([128, 512], F32R, tag=f"attc_{tagp}")
        nc.vector.tensor_copy(attc, att)
        den_ps = small_ps()
        nc.tensor.matmul(den_ps[0:4, 0:512], sel, attc, start=True, stop=True)
        den_r = sb_misc.tile([4, 512], F32R, tag=f"denr_{tagp}")
        nc.vector.reciprocal(den_r, den_ps[0:4, 0:512])
        rb_ps = small_ps()
        nc.tensor.matmul(rb_ps[:, 0:512], blk, den_r, start=True, stop=True)
        mhc = sb_misc.tile([128, 512], BF16, tag=f"mhc_{tagp}")
        nc.vector.tensor_mul(mhc, attc, rb_ps[:, 0:512])
        return mhc

    def phase_b2(st):
        b, mhc1, kts, vas, m01s, shkb, m01m = st
        mhc2 = mha_pass(b, kts, vas, m01s, S_TILES, qtz_slots[b % 2], "m2")
        mh = sb_misc.tile([128, 512], BF16, tag="mh")
        nc.vector.tensor_add(mh, mhc1, mhc2)
        cmb_ps = small_ps()
        for c in range(2):
            nc.tensor.matmul(cmb_ps[:, 0:256], wc[c],
                             mh[:, c * 256:(c + 1) * 256],
                             start=(c == 0), stop=(c == 1))
        cmbT = sb_misc.tile([128, 256], BF16, tag="cmbT")
        nc.vector.tensor_copy(cmbT, cmb_ps[:, 0:256])
        return (b, cmbT, shkb, m01m)

    def phase_c(st):
        b, cmbT, shkb, m01m = st
        w = sb_u.tile([128, MT, 256], F32R, tag="w")
        dtot = att_ps_tile()  # rows 0:1 used; shares the att PSUM ring
        for g in range(4):
            fs = small_ps()
            for j in range(2):
                m0 = (2 * g + j) * 128
                nc.tensor.matmul(fs[:, j * 256:(j + 1) * 256],
                                 shkb[:, m0:m0 + 128], cmbT,
                                 start=True, stop=True)
            th = sb_misc.tile([128, 512], FP32, tag="th")
            nc.scalar.activation(th, fs[:, 0:512], AF.Tanh,
                                 scale=float(1.0 / SQRT_E))
            nc.scalar.activation(w[:, 2 * g:2 * g + 2, :], th, AF.Exp,
                                 scale=float(CLIP))
            nc.vector.tensor_mul(w[:, 2 * g:2 * g + 2, :],
                                 w[:, 2 * g:2 * g + 2, :],
                                 m01m[:, 2 * g:2 * g + 2, :])
            for j in range(2):
                nc.tensor.matmul(dtot[0:1, 0:256], onesc,
                                 w[:, 2 * g + j, :],
                                 start=(g == 0 and j == 0),
                                 stop=(g == 3 and j == 1),
                                 skip_group_check=True)
        den_r = sb_misc.tile([1, 256], F32R, tag="denr_f")
        nc.vector.reciprocal(den_r, dtot[0:1, 0:256])
        rbf_ps = small_ps()
        nc.tensor.matmul(rbf_ps[:, 0:256], ones1r, den_r, start=True, stop=True)
        oT = sb_u.tile([128, MT, 256], FP32, tag="oT")
        rbf = rbf_ps[:, 0:256]
        rb_b = bass.AP(tensor=rbf.tensor, offset=rbf.offset,
                       ap=[rbf.ap[0], [0, MT], [1, 256]])
        nc.vector.tensor_mul(oT, w, rb_b)
        nc.sync.dma_start(out=t["outT"][b], in_=oT)

    st = phase_b2(phase_a_b1(0))
    for b in range(1, BLOC):
        half = phase_a_b1(b)
        phase_c(st)
        st = phase_b2(half)
    phase_c(st)


# ------------------------------------------------------------- host prep
def _host_arrays(inputs):
    """Layout-only host prep: transposes, pads, casts, constant columns."""
    f32 = np.float32
    eln = np.asarray(inputs["encoded_last_node"], f32)      # [B,N,E]
    load = np.asarray(inputs["load"], f32)                   # [B,N]
    solm = np.asarray(inputs["sols_mask_pomo"], f32)         # [B,N,S]
    ninf = np.asarray(inputs["ninf_mask"], f32)              # [B,N,M]
    k = np.asarray(inputs["k"], f32)                         # [B,H,M,D]
    v = np.asarray(inputs["v"], f32)
    k_s = np.asarray(inputs["k_s"], f32)                     # [B,H,S,D]
    v_s = np.asarray(inputs["v_s"], f32)
    shk = np.asarray(inputs["single_head_key"], f32)         # [B,E,M]
    wq = np.asarray(inputs["Wq_last"], f32)                  # [E, E+1]
    wcm = np.asarray(inputs["W_combine"], f32)               # [E, E]

    def tile_lp(x, pad_to):  # [B, L, C] -> [B, 128, T, C] with l = t*128+p
        Bn, L, C = x.shape
        xp = np.zeros((Bn, pad_to, C), x.dtype)
        xp[:, :L] = x
        return np.ascontiguousarray(
            xp.reshape(Bn, pad_to // 128, 128, C).transpose(0, 2, 1, 3))

    def build_va(vv, pad_to):  # [B,H,L,D] -> banded [B,128,T,256]
        Bn, Hn, L, Dn = vv.shape
        va = np.zeros((Bn, L, Hn, 32), np.float32)
        va[:, :, :, :D] = vv.transpose(0, 2, 1, 3)
        va[:, :, :, D] = 1.0
        return tile_lp(va.reshape(Bn, L, Hn * 32), pad_to).astype(np.dtype("bfloat16"))

    bf16 = np.dtype("bfloat16")
    arrs = {
        "elnT": np.ascontiguousarray(eln.transpose(0, 2, 1)),
        "loadv": np.ascontiguousarray(load),
        "ktm": np.ascontiguousarray(
            k.transpose(0, 1, 3, 2).reshape(B, H * D, M)).astype(bf16),
        "kts": np.ascontiguousarray(
            k_s.transpose(0, 1, 3, 2).reshape(B, H * D, S)).astype(bf16),
        "vam": build_va(v, MP),
        "vas": build_va(v_s, SP),
        "m01m": tile_lp((ninf == 0).astype(np.float32).transpose(0, 2, 1),
                        MP).astype(bf16),
        "m01s": tile_lp((solm == 0).astype(np.float32).transpose(0, 2, 1),
                        SP).astype(bf16),
        "shkb": np.concatenate(
            [shk, np.zeros((B, E, MP - M), f32)], axis=2).astype(bf16),
    }
    arrs["wqT"] = np.ascontiguousarray(wq[:, :E].T)
    arrs["wql"] = np.ascontiguousarray(wq[:, E:E + 1].T)
    for c in range(2):
        w = np.zeros((E, E), np.float32)
        for i in range(4):
            w[32 * i:32 * i + 16, :] = wcm[:, 64 * c + 16 * i:64 * c + 16 * i + 16].T
        arrs[f"wc{c}"] = w.astype(bf16)
    selm = np.zeros((128, 4), np.float32)
    for i in range(4):
        selm[32 * i + 16, i] = 1.0
    arrs["sel"] = selm
    blkm = np.zeros((4, 128), np.float32)
    for j in range(4):
        blkm[j, 32 * j:32 * j + 16] = 1.0
    arrs["blk"] = blkm
    arrs["onesc"] = np.ones((128, 1), np.float32)
    arrs["ones1r"] = np.ones((1, 128), np.float32)
    return arrs


_SHARED = ("wqT", "wql", "wc0", "wc1", "sel", "blk", "onesc", "ones1r")


def _in_maps(inputs):
    arrs = _host_arrays(inputs)
    in_maps = []
    for c in range(NCORES):
        s = slice(c * BLOC, (c + 1) * BLOC)
        in_maps.append({n: (a if n in _SHARED else np.ascontiguousarray(a[s]))
                        for n, a in arrs.items()})
    return in_maps


def _untranspose(outT):
    """[B,128,MT,256] -> [B,N,M] f32"""
    o = np.asarray(outT).astype(np.float32)          # [B, p, t, n]
    o = o.transpose(0, 3, 2, 1).reshape(B, N, MP)    # [B, n, t*128+p]
    return np.ascontiguousarray(o[:, :, :M])


# ------------------------------------------------------------- entry point
_NC_CACHE = None


def kernel(**inputs):
    global _NC_CACHE
    from concourse.bass_utils import run_bass_kernel_spmd

    if _NC_CACHE is None:
        _NC_CACHE = build_nc()
    nc = _NC_CACHE
    res = run_bass_kernel_spmd(nc, _in_maps(inputs), core_ids=list(range(NCORES)))
    full = np.concatenate([res.results[c]["outT"] for c in range(NCORES)], axis=0)
    return _untranspose(full)


def bench(inputs, iters=24):
    """Device-resident repeated execution; returns min wall ns per launch
    (includes PJRT dispatch, excludes H2D of inputs)."""
    import time
    import jax
    import concourse.mybir as mb
    from concourse import bass2jax
    from jax.experimental.shard_map import shard_map
    from jax.sharding import Mesh, NamedSharding, PartitionSpec

    global _NC_CACHE
    if _NC_CACHE is None:
        _NC_CACHE = build_nc()
    nc = _NC_CACHE
    bass2jax.install_neuronx_cc_hook()

    partition_name = nc.partition_id_tensor.name if nc.partition_id_tensor else None
    in_names, out_names, out_avals, zero_outs = [], [], [], []
    for alloc in nc.m.functions[0].allocations:
        if not isinstance(alloc, mb.MemoryLocationSet):
            continue
        name = alloc.memorylocations[0].name
        if alloc.kind == "ExternalInput":
            if name != partition_name:
                in_names.append(name)
        elif alloc.kind == "ExternalOutput":
            shape = tuple(alloc.tensor_shape)
            dtype = mb.dt.np(alloc.dtype)
            out_names.append(name)
            out_avals.append(jax.core.ShapedArray(shape, dtype))
            zero_outs.append(np.zeros((NCORES * shape[0], *shape[1:]), dtype))
    n_params = len(in_names)
    n_outs = len(out_avals)
    all_names = in_names + out_names + ([partition_name] if partition_name else [])
    donate = tuple(range(n_params, n_params + n_outs))

    def _body(*args):
        operands = list(args)
        if partition_name is not None:
            operands.append(bass2jax.partition_id_tensor())
        return tuple(bass2jax._bass_exec_p.bind(
            *operands, out_avals=tuple(out_avals), in_names=tuple(all_names),
            out_names=tuple(out_names), lowering_input_output_aliases=(),
            sim_require_finite=True, sim_require_nnan=True, nc=nc))

    devices = jax.devices()[:NCORES]
    mesh = Mesh(np.asarray(devices), ("core",))
    sharded = jax.jit(
        shard_map(_body, mesh=mesh,
                  in_specs=(PartitionSpec("core"),) * (n_params + n_outs),
                  out_specs=(PartitionSpec("core"),) * n_outs, check_rep=False),
        donate_argnums=donate, keep_unused=True)

    in_maps = _in_maps(inputs)
    concat_in = [np.concatenate([np.asarray(in_maps[c][nm]) for c in range(NCORES)],
                                axis=0) if in_maps[0][nm].ndim > 2 or nm not in _SHARED
                 else np.asarray(in_maps[0][nm]) for nm in in_names]
    # shared weights are identical per core: replicate along core axis
    concat_in = []
    for nm in in_names:
        if nm in _SHARED:
            a = np.asarray(in_maps[0][nm])
            concat_in.append(np.concatenate([a] * NCORES, axis=0))
        else:
            concat_in.append(np.concatenate(
                [np.asarray(in_maps[c][nm]) for c in range(NCORES)], axis=0))
    sh = NamedSharding(mesh, PartitionSpec("core"))
    dev_in = [jax.device_put(a, sh) for a in concat_in]
    times = []
    for it in range(iters):
        dev_zeros = [jax.device_put(z, sh) for z in zero_outs]
        jax.block_until_ready(dev_zeros)
        t0 = time.perf_counter()
        outs = sharded(*dev_in, *dev_zeros)
        jax.block_until_ready(outs)
        times.append(time.perf_counter() - t0)
    print(f"  launch times (ms): {[round(t*1e3, 2) for t in times]}")
    return int(min(times[1:]) * 1e9) if len(times) > 1 else int(times[0] * 1e9)


if __name__ == "__main__":
    build_nc()
    print("build ok")
